# revision 26
# baseline (speedup 1.0000x reference)
"""Contrastive-loss kernel for trn2 (8 NeuronCores, SPMD).

The reference loss reduces to a Gram matrix G = F.T @ F over the
flattened input F [N=524288, T=64], followed by a tiny [64,64] masked
margin reduction (host).  ~69us (baseline) -> ~34.5us.  Changes vs the
baseline:

1. Host-side cast fp32 -> fp8 e4m3 (ml_dtypes.float8_e4m3, the TRN
   fp8e4 format, max +-240).  Device HBM traffic drops 4x to 4.19
   MiB/core (drains in ~11.5-15us; 8 cores together sit at the chip
   HBM ceiling, so per-core rate varies 240-370 GB/s run to run);
   end-to-end loss rel-err ~7e-4 (tolerance 2e-2).  The PE matmul
   stream (~14.3us warm) and the DMA are roughly balanced.
2. PE warm-up: the HAM clock gate keeps the PE at 1.2 GHz until it has
   been busy for a full ~3.4us activity window (trace: 55 cold matmuls
   = 5.9us wasted).  A handful of junk matmuls on an uninitialized
   SBUF scratch (into a scratch PSUM bank nobody reads) fill the
   NRT-preamble -> first-tile-landed dead window so the HAM flip
   happens during warm-up, not mid-stream.
3. Packed matmuls: lhsT = rhs = [A|B] ([128,128] fp8 -> FWL),
   accumulating [[A'A,A'B],[B'A,B'B]] into one [128,128] PSUM tile;
   diagonal blocks summed by DVE (copy+add) at the end.  256 matmuls
   at ~50ns warm cadence.
4. No nc.Block(): instructions are emitted straight into the entry
   block, which removes the per-engine branch (and its ~0.9us ifetch
   stall on gpsimd right before the first DMA) and the walrus
   end-of-block barrier.
5. Minimal tail: the NRT-injected postamble zeroes ALL 256 semaphores
   (5 engines x 51 sems, trace-verified), so the kernel does no sem
   clearing of its own.  The output store's mandatory semaphore update
   (walrus crashes on a DMACopy with an empty update list) lands on
   forged low sem S[48], which nothing waits on and the postamble
   zeroes anyway -- no engine ever waits for the store's HBM write
   receipt.  gpsimd resets the SWDGE queue state after the last input
   DMA completes, off the critical path.

The 8 partial [64,64] Grams are summed on the host, where the masked
margin reduction (negligible work) also runs.
"""

import contextlib

import numpy as np
import ml_dtypes

import bass_rust
import concourse.bacc as bacc
import concourse.mybir as mybir
from concourse.bass_utils import run_bass_kernel_spmd

# See item 5 above: receipt sink for the output store.  S[48] is only
# ever touched by the NRT postamble reset chains (trace-verified).
_STORE_SEM = bass_rust.SemaphoreHandle("nrt_scratch", 48)

MARGIN = 60000.0
S = 64                           # time steps (Gram dim)
N_TOTAL = 2 * 8 * 32 * 32 * 32   # 524288 flattened rows
N_CORES = 8
N_SHARD = N_TOTAL // N_CORES     # 65536 rows per core
P = 128                          # SBUF partitions
# Tile sizes in rows (multiples of 256 so each tile is a whole number
# of packed [128,128] matmuls).  A tile's landing time has a ~1.2us
# fixed floor (per-descriptor cost, 8 descs/SDMA-engine), so tiny
# lead-in tiles land no sooner than an 8192-row tile while leaving the
# PE starved at every boundary (tiny tiles stalled the PE 1-2us per
# boundary and re-cooled its clock in an earlier rev).  The junk
# warm-up below covers the lead-in instead, and 8192-row tiles keep
# the per-tile DMA time (~1.4us) below the warm PE time per tile
# (~1.8us) so the stream does not stall mid-flight; the last tiles are
# slightly bigger for margin on HBM-contended (slow-DMA) cores.
TILE_ROWS = [8192, 8192, 8192, 8192, 8192, 12288, 12288]
# (Variants tried and rejected on hardware, all within ~1us of noise or
# worse on the worst core: a 2048/2048/4096 lead-in staircase, 4096
# tail tiles, and issuing the first tiles from the SP HWDGE ring --
# that last one cost the worst core ~3us, as mixing HWDGE and SWDGE
# queues degrades the SDMA round-robin.)
N_SYNC_TILES = 0
assert sum(TILE_ROWS) == N_SHARD and all(r % 256 == 0 for r in TILE_ROWS)
TILE_FREE = [(r // P) * S for r in TILE_ROWS]   # fp8 elems per partition
TILE_OFF = [sum(TILE_FREE[:i]) for i in range(len(TILE_ROWS))]
XBUF_FREE = sum(TILE_FREE)                      # 32768 B/partition (fp8)
N_TILES = len(TILE_ROWS)
# Junk warm-up matmuls: bridge PE-preamble-end -> tile0-landed (~3.4us)
# with N=512 matmuls (427ns each at the cold 1.2 GHz clock); by the
# time real matmuls start the HAM window has flipped to 2.4 GHz.
# (A tighter schedule -- 2048-row lead-in tiles + 5 junk MMs so real
# matmuls start ~1.2us earlier -- measured no faster and produced one
# NaN partial Gram in ~10 calls, a rare timing-dependent race; this
# configuration has a long clean record, and kernel() additionally
# verifies the partials are finite and retries once.)
N_JUNK = 8

_CACHE = {}
LAST_RESULTS = None              # BassKernelResults of the most recent run


def _build_nc():
    nc = bacc.Bacc("TRN2", target_bir_lowering=False, debug=False,
                   num_devices=N_CORES)
    # Drop the const-AP memsets and the all-engine barrier that
    # Bass.__init__ appends to the entry block (~0.5us before the first
    # kernel instruction can issue).  Nothing in this kernel uses the
    # const APs, and all cross-engine ordering is explicit via sems.
    entry = nc.main_func.blocks[0]
    first_memset = next(i for i, inst in enumerate(entry.instructions)
                        if isinstance(inst, mybir.InstMemset))
    del entry.instructions[first_memset:]

    x = nc.dram_tensor("x", [N_SHARD, S], mybir.dt.float8e4,
                       kind="ExternalInput")
    g = nc.dram_tensor("g", [S, S], mybir.dt.float32, kind="ExternalOutput")

    def tile_src(i):
        a = sum(TILE_ROWS[:i])
        return x[a:a + TILE_ROWS[i]].rearrange(
            "(p r) c -> p (r c)", p=P, r=TILE_ROWS[i] // P)

    with (
        nc.sbuf_tensor("xbuf", [P, XBUF_FREE], mybir.dt.float8e4) as xbuf,
        nc.sbuf_tensor("junk", [P, 512], mybir.dt.float8e4) as junk,
        nc.psum_tensor("acc", [2 * S, 2 * S], mybir.dt.float32) as acc,
        nc.psum_tensor("scr", [P, 512], mybir.dt.float32) as scr,
        nc.sbuf_tensor("obuf", [S, S], mybir.dt.float32) as obuf,
        nc.semaphore("pe_sem") as pe_sem,
        nc.semaphore("out_sem") as out_sem,
        contextlib.ExitStack() as stack,
    ):
        dma_sems = [stack.enter_context(nc.semaphore(f"dma_sem{k}"))
                    for k in range(N_TILES)]
        dma_lo = min(s.num for s in dma_sems)
        dma_hi = max(s.num for s in dma_sems)
        assert dma_hi - dma_lo == N_TILES - 1

        # --- SP: the first tiles via HWDGE (earliest possible start).
        for i in range(N_SYNC_TILES):
            nc.sync.dma_start(
                xbuf[:, TILE_OFF[i]:TILE_OFF[i] + TILE_FREE[i]],
                tile_src(i),
            ).then_inc(dma_sems[i], 16)
        # --- gpsimd: the bulk of the input via SWDGE, then reset the
        # SWDGE queue state.
        for i in range(N_SYNC_TILES, N_TILES):
            nc.gpsimd.dma_start(
                xbuf[:, TILE_OFF[i]:TILE_OFF[i] + TILE_FREE[i]],
                tile_src(i),
            ).then_inc(dma_sems[i], 16)
        # dma_sems[-1] == 16 implies every engine drained its FIFO
        # through the last tile, i.e. all SWDGE input DMAs completed (it
        # does NOT touch sem values, so the PE's pending per-tile waits
        # are unaffected).
        nc.gpsimd.wait_ge(dma_sems[-1], 16)
        nc.gpsimd.dma_reset(range(dma_lo, dma_hi + 1))

        # --- PE: junk warm-up (uninitialized operands, scratch PSUM,
        # nobody reads the result -- only the HAM activity matters).
        for j in range(N_JUNK):
            nc.tensor.matmul(scr[:], junk[:, :128], junk[:],
                             start=True, stop=True, skip_group_check=True)
        # --- PE: the real packed Gram accumulation.
        for i in range(N_TILES):
            nc.tensor.wait_ge(dma_sems[i], 16)
            pairs = TILE_FREE[i] // (2 * S)
            for j in range(pairs):
                c = xbuf[:, TILE_OFF[i] + j * 2 * S:
                         TILE_OFF[i] + (j + 1) * 2 * S]
                mm = nc.tensor.matmul(
                    acc[:], c, c,
                    start=(i == 0 and j == 0),
                    stop=(i == N_TILES - 1 and j == pairs - 1),
                )
                if i == N_TILES - 1 and j == pairs - 1:
                    mm.then_inc(pe_sem, 1)

        # --- DVE: merge the diagonal blocks.
        nc.vector.wait_ge(pe_sem, 1)
        nc.vector.tensor_copy(obuf[:], acc[:S, :S])
        nc.vector.tensor_add(obuf[:], obuf[:],
                             acc[S:, S:]).then_inc(out_sem, 1)

        # --- SP + ACT: store the partial Gram, half per HWDGE ring, so
        # descriptor generation (~0.6us for 64 descs) and the HBM write
        # receipt run in parallel on the two otherwise-idle sequencers.
        # Receipts land on S[48] (zeroed by the NRT postamble; nothing
        # on-device waits for them).  single_packet: the 256 B
        # descriptors sit below the 512 B SDMA line-rate threshold, so
        # concatenating each half into one packet avoids the
        # per-descriptor RMW penalty on the HBM write (the receipts
        # gate the engines' postamble drains before the end barrier).
        for eng, lo, hi in ((nc.sync, 0, S // 2), (nc.scalar, S // 2, S)):
            eng.wait_ge(out_sem, 1)
            eng.dma_start(g[lo:hi], obuf[lo:hi],
                          single_packet=True).then_inc(
                _STORE_SEM, 16, skip_validation=True)

    nc.compile()
    return nc


def get_nc():
    if "nc" not in _CACHE:
        _CACHE["nc"] = _build_nc()
    return _CACHE["nc"]


def _device_partial_grams(flat8, **run_kwargs) -> np.ndarray:
    """Run the SPMD bass kernel; return the 8 partial Grams [8, 64, 64].

    Retries once if any partial is non-finite (a rare timing-dependent
    device glitch was observed once under an aggressive DMA schedule;
    this guard costs one extra ~35us execution only when it fires).
    """
    global LAST_RESULTS
    nc = get_nc()
    in_maps = [
        {"x": flat8[c * N_SHARD:(c + 1) * N_SHARD]} for c in range(N_CORES)
    ]
    for _attempt in range(2):
        LAST_RESULTS = run_bass_kernel_spmd(
            nc, in_maps, core_ids=list(range(N_CORES)), **run_kwargs
        )
        out = np.stack([LAST_RESULTS.results[c]["g"]
                        for c in range(N_CORES)])
        if np.isfinite(out).all():
            return out
    return out


def kernel(input: np.ndarray, **run_kwargs) -> np.ndarray:
    flat = np.asarray(input, dtype=np.float32).reshape(N_TOTAL, S)
    flat8 = np.ascontiguousarray(flat.astype(ml_dtypes.float8_e4m3))
    partials = _device_partial_grams(flat8, **run_kwargs)

    gram = partials.astype(np.float64).sum(axis=0)
    sq = np.diag(gram)
    dist = sq[:, None] + sq[None, :] - 2.0 * gram
    idx = np.arange(S)
    lower = idx[:, None] > idx[None, :]
    adjacent = (idx[:, None] - idx[None, :]) == 1
    per_pair = np.where(adjacent, np.maximum(0.0, MARGIN - dist), dist)
    loss = np.where(lower, per_pair, 0.0).sum() / (S * (S - 1) * 1000)
    return np.asarray(loss, dtype=np.float32)


# revision 27
# speedup vs baseline: 1.0048x; 1.0048x over previous
"""Contrastive-loss kernel for trn2 (8 NeuronCores, SPMD).

The reference loss reduces to a Gram matrix G = F.T @ F over the
flattened input F [N=524288, T=64], followed by a tiny [64,64] masked
margin reduction (host).  ~69us (baseline) -> ~34.5us.  Changes vs the
baseline:

1. Host-side cast fp32 -> fp8 e4m3 (ml_dtypes.float8_e4m3, the TRN
   fp8e4 format, max +-240).  Device HBM traffic drops 4x to 4.19
   MiB/core (drains in ~11.5-15us; 8 cores together sit at the chip
   HBM ceiling, so per-core rate varies 240-370 GB/s run to run);
   end-to-end loss rel-err ~7e-4 (tolerance 2e-2).  The PE matmul
   stream (~14.3us warm) and the DMA are roughly balanced.
2. PE warm-up: the HAM clock gate keeps the PE at 1.2 GHz until it has
   been busy for a full ~3.4us activity window (trace: 55 cold matmuls
   = 5.9us wasted).  A handful of junk matmuls on an uninitialized
   SBUF scratch (into a scratch PSUM bank nobody reads) fill the
   NRT-preamble -> first-tile-landed dead window so the HAM flip
   happens during warm-up, not mid-stream.
3. Packed matmuls: lhsT = rhs = [A|B] ([128,128] fp8 -> FWL),
   accumulating [[A'A,A'B],[B'A,B'B]] into one [128,128] PSUM tile;
   diagonal blocks summed by DVE (copy+add) at the end.  256 matmuls
   at ~50ns warm cadence.
4. No nc.Block(): instructions are emitted straight into the entry
   block, which removes the per-engine branch (and its ~0.9us ifetch
   stall on gpsimd right before the first DMA) and the walrus
   end-of-block barrier.
5. Minimal tail: the NRT-injected postamble zeroes ALL 256 semaphores
   (5 engines x 51 sems, trace-verified), so the kernel does no sem
   clearing of its own.  The output store's mandatory semaphore update
   (walrus crashes on a DMACopy with an empty update list) lands on
   forged low sem S[48], which nothing waits on and the postamble
   zeroes anyway -- no engine ever waits for the store's HBM write
   receipt.  gpsimd resets the SWDGE queue state after the last input
   DMA completes, off the critical path.

The 8 partial [64,64] Grams are summed on the host, where the masked
margin reduction (negligible work) also runs.
"""

import contextlib

import numpy as np
import ml_dtypes

import bass_rust
import concourse.bacc as bacc
import concourse.mybir as mybir
from concourse.bass_utils import run_bass_kernel_spmd

# See item 5 above: receipt sink for the output store.  S[48] is only
# ever touched by the NRT postamble reset chains (trace-verified).
_STORE_SEM = bass_rust.SemaphoreHandle("nrt_scratch", 48)

MARGIN = 60000.0
S = 64                           # time steps (Gram dim)
N_TOTAL = 2 * 8 * 32 * 32 * 32   # 524288 flattened rows
N_CORES = 8
N_SHARD = N_TOTAL // N_CORES     # 65536 rows per core
P = 128                          # SBUF partitions
# Tile sizes in rows (multiples of 256 so each tile is a whole number
# of packed [128,128] matmuls).  A tile's landing time has a ~1.2us
# fixed floor (per-descriptor cost, 8 descs/SDMA-engine), so tiny
# lead-in tiles land no sooner than an 8192-row tile while leaving the
# PE starved at every boundary (tiny tiles stalled the PE 1-2us per
# boundary and re-cooled its clock in an earlier rev).  The junk
# warm-up below covers the lead-in instead, and 8192-row tiles keep
# the per-tile DMA time (~1.4us) below the warm PE time per tile
# (~1.8us) so the stream does not stall mid-flight; the last tiles are
# slightly bigger for margin on HBM-contended (slow-DMA) cores.
TILE_ROWS = [8192, 8192, 8192, 8192, 8192, 12288, 12288]
# (Variants tried and rejected on hardware, all within ~1us of noise or
# worse on the worst core: a 2048/2048/4096 lead-in staircase, 4096
# tail tiles, and issuing the first tiles from the SP HWDGE ring --
# that last one cost the worst core ~3us, as mixing HWDGE and SWDGE
# queues degrades the SDMA round-robin.)
N_SYNC_TILES = 0
assert sum(TILE_ROWS) == N_SHARD and all(r % 256 == 0 for r in TILE_ROWS)
TILE_FREE = [(r // P) * S for r in TILE_ROWS]   # fp8 elems per partition
TILE_OFF = [sum(TILE_FREE[:i]) for i in range(len(TILE_ROWS))]
XBUF_FREE = sum(TILE_FREE)                      # 32768 B/partition (fp8)
N_TILES = len(TILE_ROWS)
# Junk warm-up matmuls: bridge PE-preamble-end -> tile0-landed (~3.4us)
# with N=512 matmuls (427ns each at the cold 1.2 GHz clock); by the
# time real matmuls start the HAM window has flipped to 2.4 GHz.
# (A tighter schedule -- 2048-row lead-in tiles + 5 junk MMs so real
# matmuls start ~1.2us earlier -- measured no faster and produced one
# NaN partial Gram in ~10 calls, a rare timing-dependent race; this
# configuration has a long clean record, and kernel() additionally
# verifies the partials are finite and retries once.)
N_JUNK = 8

_CACHE = {}
LAST_RESULTS = None              # BassKernelResults of the most recent run


def _build_nc():
    nc = bacc.Bacc("TRN2", target_bir_lowering=False, debug=False,
                   num_devices=N_CORES)
    # Drop the const-AP memsets and the all-engine barrier that
    # Bass.__init__ appends to the entry block (~0.5us before the first
    # kernel instruction can issue).  Nothing in this kernel uses the
    # const APs, and all cross-engine ordering is explicit via sems.
    entry = nc.main_func.blocks[0]
    first_memset = next(i for i, inst in enumerate(entry.instructions)
                        if isinstance(inst, mybir.InstMemset))
    del entry.instructions[first_memset:]

    x = nc.dram_tensor("x", [N_SHARD, S], mybir.dt.float8e4,
                       kind="ExternalInput")
    g = nc.dram_tensor("g", [S, S], mybir.dt.float32, kind="ExternalOutput")

    def tile_src(i):
        a = sum(TILE_ROWS[:i])
        return x[a:a + TILE_ROWS[i]].rearrange(
            "(p r) c -> p (r c)", p=P, r=TILE_ROWS[i] // P)

    with (
        nc.sbuf_tensor("xbuf", [P, XBUF_FREE], mybir.dt.float8e4) as xbuf,
        nc.sbuf_tensor("junk", [P, 512], mybir.dt.float8e4) as junk,
        nc.psum_tensor("acc", [2 * S, 2 * S], mybir.dt.float32) as acc,
        nc.psum_tensor("scr", [P, 512], mybir.dt.float32) as scr,
        nc.sbuf_tensor("obuf", [S, S], mybir.dt.float32) as obuf,
        nc.semaphore("pe_sem") as pe_sem,
        nc.semaphore("out_sem") as out_sem,
        contextlib.ExitStack() as stack,
    ):
        dma_sems = [stack.enter_context(nc.semaphore(f"dma_sem{k}"))
                    for k in range(N_TILES)]
        dma_lo = min(s.num for s in dma_sems)
        dma_hi = max(s.num for s in dma_sems)
        assert dma_hi - dma_lo == N_TILES - 1

        # --- SP: the first tiles via HWDGE (earliest possible start).
        for i in range(N_SYNC_TILES):
            nc.sync.dma_start(
                xbuf[:, TILE_OFF[i]:TILE_OFF[i] + TILE_FREE[i]],
                tile_src(i),
            ).then_inc(dma_sems[i], 16)
        # --- gpsimd: the bulk of the input via SWDGE, then reset the
        # SWDGE queue state.
        for i in range(N_SYNC_TILES, N_TILES):
            nc.gpsimd.dma_start(
                xbuf[:, TILE_OFF[i]:TILE_OFF[i] + TILE_FREE[i]],
                tile_src(i),
            ).then_inc(dma_sems[i], 16)
        # dma_sems[-1] == 16 implies every engine drained its FIFO
        # through the last tile, i.e. all SWDGE input DMAs completed (it
        # does NOT touch sem values, so the PE's pending per-tile waits
        # are unaffected).
        nc.gpsimd.wait_ge(dma_sems[-1], 16)
        nc.gpsimd.dma_reset(range(dma_lo, dma_hi + 1))

        # --- PE: junk warm-up (uninitialized operands, scratch PSUM,
        # nobody reads the result -- only the HAM activity matters).
        for j in range(N_JUNK):
            nc.tensor.matmul(scr[:], junk[:, :128], junk[:],
                             start=True, stop=True, skip_group_check=True)
        # --- PE: the real packed Gram accumulation.
        for i in range(N_TILES):
            nc.tensor.wait_ge(dma_sems[i], 16)
            pairs = TILE_FREE[i] // (2 * S)
            for j in range(pairs):
                c = xbuf[:, TILE_OFF[i] + j * 2 * S:
                         TILE_OFF[i] + (j + 1) * 2 * S]
                mm = nc.tensor.matmul(
                    acc[:], c, c,
                    start=(i == 0 and j == 0),
                    stop=(i == N_TILES - 1 and j == pairs - 1),
                )
                if i == N_TILES - 1 and j == pairs - 1:
                    mm.then_inc(pe_sem, 1)

        # --- DVE: merge the diagonal blocks.
        nc.vector.wait_ge(pe_sem, 1)
        nc.vector.tensor_copy(obuf[:], acc[:S, :S])
        nc.vector.tensor_add(obuf[:], obuf[:],
                             acc[S:, S:]).then_inc(out_sem, 1)

        # --- SP: store the partial Gram (receipt lands on S[48],
        # zeroed by the NRT postamble; nothing on-device waits for it).
        # single_packet: the store's 64 descriptors are 256 B each --
        # below the 512 B SDMA line-rate threshold -- so concatenating
        # them into one packet avoids the per-descriptor RMW penalty on
        # the HBM write (the receipt gates SP's postamble drain).
        # (Splitting the store across the SP and ACT rings was tried:
        # parallel descriptor-gen and receipts, but it measured ~0.7us
        # WORSE -- first use of the ACT HWDGE ring is expensive.)
        nc.sync.wait_ge(out_sem, 1)
        nc.sync.dma_start(g[:], obuf[:], single_packet=True).then_inc(
            _STORE_SEM, 16, skip_validation=True)

    nc.compile()
    return nc


def get_nc():
    if "nc" not in _CACHE:
        _CACHE["nc"] = _build_nc()
    return _CACHE["nc"]


def _device_partial_grams(flat8, **run_kwargs) -> np.ndarray:
    """Run the SPMD bass kernel; return the 8 partial Grams [8, 64, 64].

    Retries once if any partial is non-finite (a rare timing-dependent
    device glitch was observed once under an aggressive DMA schedule;
    this guard costs one extra ~35us execution only when it fires).
    """
    global LAST_RESULTS
    nc = get_nc()
    in_maps = [
        {"x": flat8[c * N_SHARD:(c + 1) * N_SHARD]} for c in range(N_CORES)
    ]
    for _attempt in range(2):
        LAST_RESULTS = run_bass_kernel_spmd(
            nc, in_maps, core_ids=list(range(N_CORES)), **run_kwargs
        )
        out = np.stack([LAST_RESULTS.results[c]["g"]
                        for c in range(N_CORES)])
        if np.isfinite(out).all():
            return out
    return out


def kernel(input: np.ndarray, **run_kwargs) -> np.ndarray:
    flat = np.asarray(input, dtype=np.float32).reshape(N_TOTAL, S)
    flat8 = np.ascontiguousarray(flat.astype(ml_dtypes.float8_e4m3))
    partials = _device_partial_grams(flat8, **run_kwargs)

    gram = partials.astype(np.float64).sum(axis=0)
    sq = np.diag(gram)
    dist = sq[:, None] + sq[None, :] - 2.0 * gram
    idx = np.arange(S)
    lower = idx[:, None] > idx[None, :]
    adjacent = (idx[:, None] - idx[None, :]) == 1
    per_pair = np.where(adjacent, np.maximum(0.0, MARGIN - dist), dist)
    loss = np.where(lower, per_pair, 0.0).sum() / (S * (S - 1) * 1000)
    return np.asarray(loss, dtype=np.float32)
